# revision 52
# baseline (speedup 1.0000x reference)
"""Dense-CRF relaxed Potts loss on 8 TRN2 NeuronCores — fp8 DoubleRow version.

v3: fp8e4m3 DoubleRow z-matmul (0.5 PE cycles/row), log s_i folded into the
matmul so activations need no per-slab bias and groups can span slabs
(fewer, larger ACT instructions), and direct row-sums load-balanced between
ACT accum_out and a PE ones-contraction into a second PSUM accumulator.

Math: loss*N = sum_ij s_i W_ij (1-s_j).  72 slabs of 128 rows; core k owns
slabs {k+8t}. Column data is rotated by k slabs so the SPMD program is
core-independent. Off-diagonal slab pairs are processed once (offsets
d=1..35 relative to the owning slab): the 'direct' term s_i W (1-s_j) comes
straight out of T = exp(z); the mirrored term (1-s_i) W s_j = h_i T_ij r_j
is computed as a DVE elementwise T*R followed by a PE contraction with the
2-limb h against a [2,512] PSUM accumulator (column-folded, partitions 0-1
of the 8th PSUM bank). Direct sums either ride the activation's accum_out
or a PE ones-contraction folded into partition 32 of the same bank. d=0 and
d=36 blocks are direct-only (d=36 appears once in each owning slab's run,
so both orientations are covered).

z = -0.5*d2 + log s_i + log1p(-s_j) is computed as a K=98 fp8e4m3
limb-pair decomposition (4 limbs/feature, cross pairs li+lj<=5, per-row
pow2 balancing so no limb under/overflows fp8; per-dim row ordering keeps
PE partial sums small for near pairs, where exp matters). fp32-grade z at
0.5 cycles/row via MatmulPerfMode.DoubleRow ([Kp=49, 2, .] layout).
"""

import numpy as np
import ml_dtypes

import concourse.bacc as bacc
import concourse.tile as tile
from concourse import mybir
import concourse.bass_utils as bass_utils

F8 = ml_dtypes.float8_e4m3fn
BF16 = ml_dtypes.bfloat16

SIGMA_XY = 15.0
SIGMA_RGB = 0.125
H = W = 96
N = H * W                   # 9216
N_CORES = 8
NSLAB = N // 128            # 72 slabs of 128 rows
T_SLABS = NSLAB // N_CORES  # 9 per core
D_MAX = 36
NLIMB = 4
KP = 49                     # fp8 row pairs: K=98
PSA = 16                    # psA pool blocks (4 banks)
PSB = 12                    # psB pool blocks (3 banks)
HEAD_SIZES = (4, 8)         # small first groups for a fast start

_cached = {}


def _stream():
    """Block stream: (t, m, mirror) with m the local column-slab index.
    Mirror blocks (d=1..35) slab-major first, then the 18 direct-only
    blocks (d=0 and d=36 of each slab) at the tail."""
    out = []
    for t in range(T_SLABS):
        for d in range(1, D_MAX):
            out.append((t, (8 * t + d) % NSLAB, True))
    for t in range(T_SLABS):
        out.append((t, 8 * t, False))
        out.append((t, (8 * t + D_MAX) % NSLAB, False))
    return out


def _plan():
    """Compile-time schedule.

    Returns list of group dicts:
      size, parity, zchunks [(o, nb, t, m0)], mpieces [(o, nb, t, m0)]
    where o/nb are block offsets/counts inside the group. zchunks respect
    PSUM bank alignment (cannot cross o%4 boundaries) and column/slab
    contiguity; mpieces are maximal same-slab column-contiguous mirror
    runs (SBUF-side, no bank constraint).
    """
    stream = _stream()
    groups = []
    i = 0
    parity = 0
    while i < len(stream):
        cap = (HEAD_SIZES[len(groups)] if len(groups) < len(HEAD_SIZES)
               else (PSA, PSB)[parity])
        nb = min(cap, len(stream) - i)
        blocks = stream[i:i + nb]

        def contiguous(x, y):
            return (x[0] == y[0] and x[2] == y[2]
                    and y[1] == x[1] + 1 and y[1] != 0)

        zchunks = []
        j = 0
        while j < nb:
            j2 = j + 1
            while (j2 < nb and j2 % 4 != 0
                   and contiguous(blocks[j2 - 1], blocks[j2])):
                j2 += 1
            zchunks.append((j, j2 - j, blocks[j][0], blocks[j][1]))
            j = j2
        mpieces = []
        j = 0
        while j < nb:
            if not blocks[j][2]:
                j += 1
                continue
            j2 = j + 1
            while (j2 < nb and blocks[j2][2]
                   and contiguous(blocks[j2 - 1], blocks[j2])):
                j2 += 1
            mpieces.append((j, j2 - j, blocks[j][0], blocks[j][1]))
            j = j2
        groups.append(dict(size=nb, parity=parity, zchunks=zchunks,
                           mpieces=mpieces))
        i += nb
        parity ^= 1
    return groups


def _routes(groups):
    """Direct-sum route per group: 'act' (accum_out) or 'pe' (ones
    contraction into M23 partition 32). PE takes mid-stream groups; ACT
    keeps the head (PE is cold) and the tail (short epilogue)."""
    n = len(groups)
    routes = []
    for gi in range(n):
        if gi < 2 or gi >= n - 3:
            routes.append("act")
        elif gi == 2 or 5 <= gi <= 16:
            routes.append("pool")
        else:
            routes.append("pe")
    return routes


def _build_module():
    groups = _plan()
    routes = _routes(groups)
    n_groups = len(groups)

    n_mir = sum(-(-p[1] * 128 // 512) for g in groups for p in g["mpieces"])
    n_ones = sum(-(-g["size"] * 128 // 512)
                 for gi, g in enumerate(groups) if routes[gi] == "pe")

    nc = bacc.Bacc(
        "TRN2",
        target_bir_lowering=False,
        debug=False,
        enable_asserts=False,
        num_devices=N_CORES,
    )
    f32 = mybir.dt.float32
    bf = mybir.dt.bfloat16
    f8 = mybir.dt.float8e4

    ab0_src = nc.dram_tensor("ab0_src", [KP, 2, 1664], f8,
                             kind="ExternalInput").ap()
    a_src = nc.dram_tensor("a_src", [KP, 2, T_SLABS * 128], f8,
                           kind="ExternalInput").ap()
    b_src = nc.dram_tensor("b_src", [KP, 2, N], f8, kind="ExternalInput").ap()
    r_src = nc.dram_tensor("r_src", [1, N], bf, kind="ExternalInput").ap()
    h_src = nc.dram_tensor("h_src", [128, 2 * T_SLABS], bf,
                           kind="ExternalInput").ap()
    accd_out = nc.dram_tensor("accd_out", [128, n_groups], f32,
                              kind="ExternalOutput").ap()
    m23_out = nc.dram_tensor("m23_out", [2, 512], f32,
                             kind="ExternalOutput").ap()
    m3_out = nc.dram_tensor("m3_out", [1, 512], f32,
                            kind="ExternalOutput").ap()
    dsum_out = nc.dram_tensor("dsum_out", [1, n_groups], f32,
                              kind="ExternalOutput").ap()

    with tile.TileContext(nc) as tc:
        with (
            tc.tile_pool(name="singles", bufs=1) as singles,
            tc.tile_pool(name="psA", bufs=1, space="PSUM") as psA_pool,
            tc.tile_pool(name="psB", bufs=1, space="PSUM") as psB_pool,
            tc.tile_pool(name="m23ps", bufs=1, space="PSUM") as m23_pool,
            tc.tile_pool(name="tpool", bufs=12) as t_pool,
            tc.tile_pool(name="trpool", bufs=4) as tr_pool,
        ):
            AB0 = singles.tile([KP, 2, 1664], f8)
            A = singles.tile([KP, 2, T_SLABS * 128], f8)
            B = singles.tile([KP, 2, N], f8)
            R = singles.tile([128, N], bf)
            Hh = singles.tile([128, 2 * T_SLABS], bf)
            ONES = singles.tile([128, 1], bf)
            ACCD = singles.tile([128, n_groups], f32)
            M23 = m23_pool.tile([33, 512], f32)
            M23S = singles.tile([2, 512], f32)
            M3S = singles.tile([1, 512], f32)
            DSUMP = singles.tile([1, n_groups], f32)

            # trigger the ACT exp table load immediately
            DUM = singles.tile([128, 1], f32)
            nc.gpsimd.memset(DUM[:], 0.0)
            nc.gpsimd.memset(ONES[:], 1.0)
            nc.gpsimd.memset(ACCD[:], 0.0)
            nc.gpsimd.memset(DSUMP[:], 0.0)
            nc.scalar.activation(
                DUM[:], DUM[:], mybir.ActivationFunctionType.Exp,
                bias=0.0, scale=0.0,
            )
            # staged DMAs: B chunks stream on the SP queue in consumption
            # order; A/R/Hh go via the Pool (SWDGE) queue in parallel so
            # the head isn't serialized on one sequencer.
            nc.sync.dma_start(AB0[:], ab0_src)
            nc.sync.dma_start(B[:, :, 128:1664], b_src[:, :, 128:1664])
            nc.sync.dma_start(B[:, :, 1664:3712], b_src[:, :, 1664:3712])
            nc.sync.dma_start(Hh[:], h_src)
            nc.sync.dma_start(B[:, :, 3712:4608], b_src[:, :, 3712:4608])
            nc.sync.dma_start(B[:, :, 4608:6656], b_src[:, :, 4608:6656])
            nc.sync.dma_start(B[:, :, 6656:N], b_src[:, :, 6656:N])
            nc.sync.dma_start(B[:, :, 0:128], b_src[:, :, 0:128])
            nc.gpsimd.dma_start(A[:], a_src)
            nc.gpsimd.dma_start(R[:, 128:1664],
                                r_src[:, 128:1664].broadcast_to((128, 1536)))
            nc.gpsimd.dma_start(R[:, 1664:4608],
                                r_src[:, 1664:4608].broadcast_to((128, 2944)))
            nc.gpsimd.dma_start(R[:, 4608:6912],
                                r_src[:, 4608:6912].broadcast_to((128, 2304)))
            nc.gpsimd.dma_start(R[:, 6912:N],
                                r_src[:, 6912:N].broadcast_to((128, 2304)))
            nc.gpsimd.dma_start(R[:, 0:128],
                                r_src[:, 0:128].broadcast_to((128, 128)))

            mir_i = 0
            ones_i = 0
            pts = {}
            t_tiles = {}

            def emit_z(gi):
                g = groups[gi]
                parity = g["parity"]
                pool_g = psA_pool if parity == 0 else psB_pool
                cap = (PSA, PSB)[parity]
                pt = pool_g.tile([128, cap * 128], f32, tag=f"ps{parity}")
                pts[gi] = pt
                for (o, nb, t, m0) in g["zchunks"]:
                    q0, qw, c0 = o * 128, nb * 128, m0 * 128
                    if gi <= 1:
                        lhsT, rhs = AB0[:, :, 0:128], AB0[:, :, c0:c0 + qw]
                    else:
                        lhsT = A[:, :, t * 128:(t + 1) * 128]
                        rhs = B[:, :, c0:c0 + qw]
                    nc.tensor.matmul(
                        pt[:, q0:q0 + qw],
                        lhsT=lhsT,
                        rhs=rhs,
                        start=True, stop=True,
                        perf_mode=mybir.MatmulPerfMode.DoubleRow,
                    )

            def emit_act(gi):
                g = groups[gi]
                width = g["size"] * 128
                T = t_pool.tile([128, PSA * 128], bf, tag="T")
                t_tiles[gi] = T
                accum = ACCD[:, gi:gi + 1] if routes[gi] == "act" else None
                nc.scalar.activation(
                    T[:, 0:width], pts.pop(gi)[:, 0:width],
                    mybir.ActivationFunctionType.Exp,
                    bias=0.0, scale=1.0,
                    accum_out=accum,
                )

            def emit_tail(gi):
                nonlocal mir_i, ones_i
                g = groups[gi]
                width = g["size"] * 128
                T = t_tiles.pop(gi)
                for (o, nb, t, m0) in g["mpieces"]:
                    o0, w, c0 = o * 128, nb * 128, m0 * 128
                    TR = tr_pool.tile([128, PSA * 128], bf, tag="TR")
                    nc.vector.tensor_tensor(
                        TR[:, o0:o0 + w], T[:, o0:o0 + w],
                        R[:, c0:c0 + w], mybir.AluOpType.mult,
                    )
                    for q in range(0, w, 512):
                        qw = min(512, w - q)
                        nc.tensor.matmul(
                            M23[0:2, 0:qw],
                            lhsT=Hh[:, 2 * t:2 * t + 2],
                            rhs=TR[:, o0 + q:o0 + q + qw],
                            start=(mir_i == 0),
                            stop=(mir_i == n_mir - 1),
                            skip_group_check=True,
                        )
                        mir_i += 1
                if routes[gi] == "pe":
                    for q in range(0, width, 512):
                        qw = min(512, width - q)
                        nc.tensor.matmul(
                            M23[32:33, 0:qw],
                            lhsT=ONES[:],
                            rhs=T[:, q:q + qw],
                            start=(ones_i == 0),
                            stop=(ones_i == n_ones - 1),
                            skip_group_check=True,
                            tile_position=(0, 32),
                        )
                        ones_i += 1
                elif routes[gi] == "dve":
                    nc.vector.tensor_reduce(
                        ACCD[:, gi:gi + 1], T[:, 0:width],
                        mybir.AxisListType.X, mybir.AluOpType.add,
                    )
                elif routes[gi] == "pool":
                    nc.gpsimd.tensor_reduce(
                        DSUMP[0:1, gi:gi + 1], T[:, 0:width],
                        mybir.AxisListType.XYZWC, mybir.AluOpType.add,
                    )

            # software-pipelined emission: z(g+2) goes to the PE queue
            # right after ACT(g) so PE never parks mirror work in front
            # of the next group's PSUM fill.
            m3_done = [False]
            emit_z(0)
            emit_z(1)
            for gi in range(n_groups):
                emit_act(gi)
                if gi + 2 < n_groups:
                    emit_z(gi + 2)
                emit_tail(gi)
                if gi == n_groups // 2:
                    nc.sync.dma_start(accd_out[:, 0:gi], ACCD[:, 0:gi])
                if ones_i == n_ones and not m3_done[0]:
                    m3_done[0] = True
                    nc.vector.tensor_copy(M3S[:], M23[32:33, :])
                    nc.sync.dma_start(m3_out, M3S[:])


            assert mir_i == n_mir and ones_i == n_ones
            half = n_groups // 2
            nc.vector.tensor_copy(M23S[:], M23[0:2, :])
            nc.sync.dma_start(dsum_out, DSUMP[:])
            nc.sync.dma_start(m23_out, M23S[:])
            nc.sync.dma_start(accd_out[:, half:n_groups - 1],
                              ACCD[:, half:n_groups - 1])
            nc.scalar.dma_start(accd_out[:, n_groups - 1:n_groups],
                                ACCD[:, n_groups - 1:n_groups])

    nc.compile()
    return nc, routes, n_groups


def _limbs_f8(x, n=NLIMB):
    x = np.asarray(x, np.float64)
    out = []
    r = x
    for _ in range(n):
        l = r.astype(F8)
        out.append(l)
        r = r - l.astype(np.float64)
    return out


def _limbs2(x):
    x = np.asarray(x, np.float64)
    l1 = x.astype(BF16)
    l2 = (x - l1.astype(np.float64)).astype(BF16)
    return l1, l2


def _build_rows(feat, s):
    """fp8 limb-pair rows for z = -0.5*d2 + log s_i + log1p(-s_j).
    Per-dim interleave keeps PE partial sums small for near pairs."""
    sq_d = feat * feat
    s64 = np.asarray(s, np.float64)
    with np.errstate(divide="ignore"):
        logs = np.maximum(np.log(s64), -500.0)
        lp = np.maximum(np.log1p(-s64), -500.0)
    a_rows, b_rows = [], []
    pairs = [(i, j) for i in range(1, NLIMB + 1) for j in range(1, NLIMB + 1)
             if i + j <= NLIMB + 1]
    ones = np.ones(N, np.float64)
    two = (ones * 2.0).astype(F8)
    for d in range(5):
        for l in _limbs_f8(-0.5 * sq_d[:, d] * 0.5):
            a_rows.append(two)
            b_rows.append(l)
        fl = _limbs_f8(feat[:, d])
        for (li, lj) in pairs:
            p = 2 * (lj - li)
            a_rows.append((fl[li - 1].astype(np.float64) * 2.0**-p).astype(F8))
            b_rows.append((fl[lj - 1].astype(np.float64) * 2.0**p).astype(F8))
        for l in _limbs_f8(-0.5 * sq_d[:, d] * 0.5):
            a_rows.append(l)
            b_rows.append(two)
    for l in _limbs_f8(logs * 0.5):
        a_rows.append(l)
        b_rows.append(two)
    for l in _limbs_f8(lp * 0.5):
        a_rows.append(two)
        b_rows.append(l)
    a = np.stack(a_rows)
    b = np.stack(b_rows)
    assert a.shape[0] == 2 * KP, a.shape
    return a, b


def _prep_inputs(input, image):
    s = np.asarray(input, np.float32).reshape(N).astype(np.float64)
    img = np.asarray(image, np.float32).reshape(3, N).astype(np.float64)
    yy, xx = np.meshgrid(
        np.arange(H, dtype=np.float64), np.arange(W, dtype=np.float64),
        indexing="ij")
    pos = np.stack([xx, yy], -1).reshape(N, 2) / SIGMA_XY
    feat = np.concatenate([pos, img.T / SIGMA_RGB], 1)

    a_all, b_all = _build_rows(feat, s)     # [2*KP, N] fp8

    r_full = np.minimum(s / np.maximum(1.0 - s, 1e-300), 1e30).astype(BF16)
    h_full = np.minimum((1.0 - s) / np.maximum(s, 1e-300), 1e30)

    in_maps = []
    for k in range(N_CORES):
        own = [(k + 8 * t) % NSLAB for t in range(T_SLABS)]
        rot = [(k + m) % NSLAB for m in range(NSLAB)]
        rows = np.concatenate(
            [np.arange(a0 * 128, (a0 + 1) * 128) for a0 in own])
        cols = np.concatenate(
            [np.arange(m0 * 128, (m0 + 1) * 128) for m0 in rot])
        h1, h2 = _limbs2(h_full[rows])
        h_packed = np.stack(
            [h1.reshape(T_SLABS, 128), h2.reshape(T_SLABS, 128)], 1)
        h_arr = np.ascontiguousarray(
            h_packed.reshape(T_SLABS * 2, 128).T.astype(BF16))
        a_k = a_all[:, rows].reshape(KP, 2, T_SLABS * 128)
        b_k = b_all[:, cols].reshape(KP, 2, N)
        in_maps.append(
            {
                "ab0_src": np.ascontiguousarray(
                    np.concatenate([a_k[:, :, 0:128], b_k[:, :, 128:1664]],
                                   axis=2)),
                "a_src": np.ascontiguousarray(
                    a_all[:, rows].reshape(KP, 2, T_SLABS * 128)),
                "b_src": np.ascontiguousarray(
                    b_all[:, cols].reshape(KP, 2, N)),
                "r_src": np.ascontiguousarray(r_full[cols])[None, :],
                "h_src": h_arr,
            }
        )
    return in_maps


def _get_module():
    if "nc" not in _cached:
        _cached["nc"], _cached["routes"], _cached["n_groups"] = \
            _build_module()
    return _cached["nc"], _cached["routes"], _cached["n_groups"]


def _run(in_maps, **kwargs):
    nc, _, _ = _get_module()
    return bass_utils.run_bass_kernel_spmd(
        nc, in_maps, core_ids=list(range(N_CORES)), **kwargs
    )


def kernel(input, image):
    assert input.shape == (1, 1, H, W) and image.shape == (1, 3, H, W)
    nc, routes, n_groups = _get_module()
    in_maps = _prep_inputs(input, image)
    res = _run(in_maps)
    act_cols = [gi for gi in range(n_groups) if routes[gi] in ("act", "dve")]
    pool_cols = [gi for gi in range(n_groups) if routes[gi] == "pool"]
    total = 0.0
    for k in range(N_CORES):
        r = res.results[k]
        total += r["accd_out"][:, act_cols].sum(dtype=np.float64)
        total += r["dsum_out"][0, pool_cols].sum(dtype=np.float64)
        total += r["m23_out"].sum(dtype=np.float64)
        total += r["m3_out"].sum(dtype=np.float64)
    return np.array(total / N, dtype=np.float32)
